# revision 4
# baseline (speedup 1.0000x reference)
"""GQA attention kernel for 8 trn2 NeuronCores.

Sharding: core = (b, h) with b = core//4 (batch), h = core%4 (kv head).
Each core handles q heads 4h..4h+3 (a contiguous 512-column block of Wq),
its own kv head (128 rows of Wk/Wv), and the matching 512-column slice of
Wo.  Per-core output is a partial y (row-parallel Wo); host sums the 4
fp16 partials per batch in fp32.

All matmuls run in fp16 (full-rate, half the SBUF/DMA traffic of fp32r)
with fp32 PSUM accumulation.  Host pre-transposes x and the weight
shards so the device never transposes activations; only vT -> V (16
tiles) uses PE transpose.

Order per core: k proj, v proj (+transpose), then per q head: q proj
followed by that head's attention (so scalar-engine exp overlaps tensor
work of the next head), then the output projection.
"""

import numpy as np

EMB = 2048
N = 2048          # sequence length
HD = 128          # head dim
NHC = 4           # q heads per core
DQ = NHC * HD     # 512: per-core q concat dim
EC = 16           # e chunks of 128
SC = 16           # s chunks of 128
NB = 512          # n block in projection phase
M = 1024          # n-half size in attention phase
SCALE = 1.0 / np.sqrt(HD)

_NC = None


def _build():
    import concourse.bass as bass
    from concourse import bacc
    import concourse.mybir as mybir
    import concourse.tile as tile
    from concourse.bass import ts

    FP32 = mybir.dt.float32
    F16 = mybir.dt.float16
    P = 128

    nc = bacc.Bacc("TRN2", target_bir_lowering=False, debug=False, num_devices=8)
    xT = nc.declare_dram_parameter("xT", [EMB, N], F16, isOutput=False)
    wqT = nc.declare_dram_parameter("wqT", [EMB, DQ], F16, isOutput=False)
    wkT = nc.declare_dram_parameter("wkT", [EMB, HD], F16, isOutput=False)
    wvT = nc.declare_dram_parameter("wvT", [EMB, HD], F16, isOutput=False)
    woT = nc.declare_dram_parameter("woT", [DQ, EMB], F16, isOutput=False)
    iden_d = nc.declare_dram_parameter("iden", [128, 128], F16, isOutput=False)
    ones_d = nc.declare_dram_parameter("ones", [128, 1], F16, isOutput=False)
    y = nc.declare_dram_parameter("y", [N, EMB], F16, isOutput=True)

    xT_r = xT[:].rearrange("(c p) n -> p c n", p=P)      # (128, 16, 2048)
    wqT_r = wqT[:].rearrange("(c p) d -> p c d", p=P)    # (128, 16, 512)
    wkT_r = wkT[:].rearrange("(c p) d -> p c d", p=P)    # (128, 16, 128)
    wvT_r = wvT[:].rearrange("(c p) d -> p c d", p=P)
    woT_r = woT[:].rearrange("(c p) e -> p c e", p=P)    # (128, 4, 2048)

    with tile.TileContext(nc) as tc:
      with tc.tile_pool(name="consts", bufs=1) as consts, \
           tc.tile_pool(name="persist", bufs=1) as persist:
        identity = consts.tile([P, P], F16, tag="identity")
        ones = consts.tile([P, 1], F16, tag="ones")
        # small weights first so the k projection can start right away
        wk = persist.tile([P, EC, HD], F16, tag="wk")
        wv = persist.tile([P, EC, HD], F16, tag="wv")
        nc.sync.dma_start(wk[:], wkT_r)
        nc.sync.dma_start(wv[:], wvT_r)
        nc.sync.dma_start(identity[:], iden_d[:])
        nc.sync.dma_start(ones[:], ones_d[:])

        # full x stays resident in SBUF (64 KiB/partition); DMA'd in
        # n-chunks so the k projection can start after the first chunk
        xt = persist.tile([P, EC, N], F16, tag="xt")
        for nb in range(N // NB):
            nc.sync.dma_start(xt[:, :, ts(nb, NB)], xT_r[:, :, ts(nb, NB)])

        wq = persist.tile([P, EC, DQ], F16, tag="wq")
        nc.sync.dma_start(wq[:, :, 0:256], wqT_r[:, :, 0:256])
        nc.sync.dma_start(wq[:, :, 256:512], wqT_r[:, :, 256:512])
        wo = persist.tile([P, NHC, EMB], F16, tag="wo")
        nc.sync.dma_start(wo[:], woT_r)

        kT = persist.tile([P, N], F16, tag="kT")
        V = persist.tile([P, SC, HD], F16, tag="V")
        qT = [persist.tile([P, N], F16, tag=f"qT{g}", name=f"qT{g}")
              for g in range(NHC)]
        OT = [persist.tile([P, N], F16, tag=f"OT{g}", name=f"OT{g}")
              for g in range(NHC)]

        # ---------------- k/v projections ----------------
        with tc.tile_pool(name="vTp", bufs=1) as vTp, \
             tc.tile_pool(name="psA", bufs=4, space="PSUM") as psA, \
             tc.tile_pool(name="psT", bufs=2, space="PSUM") as psT:
            vT = vTp.tile([P, N], F16, tag="vT")
            for nb in range(N // NB):
                nsl = ts(nb, NB)
                for t in range(2):
                    ps = psA.tile([P, NB], FP32, tag="psA", name=f"psKV_{nb}_{t}")
                    w = wk if t == 0 else wv
                    for e in range(EC):
                        nc.tensor.matmul(
                            ps[:], w[:, e, :], xt[:, e, nsl],
                            start=(e == 0), stop=(e == EC - 1),
                        )
                    if t == 0:
                        nc.scalar.copy(kT[:, nsl], ps[:])
                    else:
                        nc.scalar.copy(vT[:, nsl], ps[:])
                # transpose the 4 freshly-written vT s-chunks into V
                for j in range(nb * 4, nb * 4 + 4):
                    pt = psT.tile([P, P], F16, tag="psT", name=f"psT_{j}")
                    nc.tensor.transpose(pt[:], vT[:, ts(j, P)], identity[:])
                    nc.scalar.copy(V[:, j, :], pt[:])

        # ---------------- per-head: q proj then attention ----------------
        with tc.tile_pool(name="esp", bufs=3) as esp, \
             tc.tile_pool(name="lap", bufs=2) as lap, \
             tc.tile_pool(name="rp", bufs=2) as rp, \
             tc.tile_pool(name="rbp", bufs=2) as rbp, \
             tc.tile_pool(name="psS", bufs=2, space="PSUM") as psS, \
             tc.tile_pool(name="psO", bufs=2, space="PSUM") as psO:
            for g in range(NHC):
                # q projection for head g (2 n-blocks of 1024 x 16 e-chunks);
                # shares the psS buffers (same tag/shape) to fit PSUM
                for nb in range(N // M):
                    ps = psS.tile([P, M], FP32, tag="psS", name=f"psQ_{g}_{nb}")
                    for e in range(EC):
                        for u in range(2):
                            nc.tensor.matmul(
                                ps[:, ts(u, NB)],
                                wq[:, e, ts(g, HD)],
                                xt[:, e, ts(2 * nb + u, NB)],
                                start=(e == 0), stop=(e == EC - 1),
                            )
                    nc.scalar.copy(qT[g][:, ts(nb, M)], ps[:])

                # attention for head g over two n-halves
                for m in range(2):
                    msl = ts(m, M)
                    lacc = lap.tile([P, M], F16, tag="lacc",
                                    name=f"lacc_{g}_{m}")
                    ot_ps = psO.tile([P, M], FP32, tag="psO",
                                     name=f"psO_{g}_{m}")
                    for j in range(SC):
                        s_ps = psS.tile([P, M], FP32, tag="psS",
                                        name=f"psS_{g}_{m}_{j}")
                        for u in range(2):
                            nc.tensor.matmul(
                                s_ps[:, ts(u, 512)],
                                kT[:, ts(j, P)],
                                qT[g][:, ts(2 * m + u, 512)],
                                start=True, stop=True,
                            )
                        es = esp.tile([P, M], F16, tag="es",
                                      name=f"es_{g}_{m}_{j}")
                        nc.scalar.activation(
                            es[:], s_ps[:],
                            mybir.ActivationFunctionType.Exp,
                            scale=float(SCALE),
                        )
                        if j == 0:
                            nc.vector.tensor_copy(lacc[:], es[:])
                        else:
                            nc.vector.tensor_add(lacc[:], lacc[:], es[:])
                        for u in range(2):
                            nc.tensor.matmul(
                                ot_ps[:, ts(u, 512)],
                                V[:, j, :],
                                es[:, ts(u, 512)],
                                start=(j == 0), stop=(j == SC - 1),
                            )
                    # partition-reduce lacc via ones-matmul -> (1, M)
                    psl = psS.tile([1, M], FP32, tag="psS",
                                   name=f"psl_{g}_{m}")
                    for u in range(2):
                        nc.tensor.matmul(
                            psl[:, ts(u, 512)],
                            ones[:, 0:1],
                            lacc[:, ts(u, 512)],
                            start=True, stop=True,
                        )
                    # broadcast THEN reciprocal so the DVE runs full-width
                    r_t = rp.tile([1, M], FP32, tag="r", name=f"r_{g}_{m}")
                    nc.scalar.copy(r_t[:], psl[:])
                    rb = rbp.tile([P, M], FP32, tag="rb", name=f"rb_{g}_{m}")
                    nc.gpsimd.partition_broadcast(rb[:], r_t[:])
                    nc.vector.reciprocal(rb[:], rb[:])
                    nc.vector.tensor_mul(OT[g][:, msl], ot_ps[:], rb[:])

        # ---------------- output projection ----------------
        with tc.tile_pool(name="yep", bufs=2) as yep, \
             tc.tile_pool(name="psC", bufs=2, space="PSUM") as psC:
            for nt in range(N // P):
                yp = psC.tile([P, EMB], FP32, tag="psC", name=f"psC_{nt}")
                for g in range(NHC):
                    lhsT = OT[g][:, ts(nt, P)]
                    for ob in range(4):
                        nc.tensor.matmul(
                            yp[:, ts(ob, 512)],
                            lhsT,
                            wo[:, g, ts(ob, 512)],
                            start=(g == 0), stop=(g == NHC - 1),
                        )
                ysb = yep.tile([P, EMB], F16, tag="ysb", name=f"ysb_{nt}")
                nc.scalar.copy(ysb[:], yp[:])
                nc.sync.dma_start(y[ts(nt, P), :], ysb[:])

    nc.compile()
    return nc


def _in_maps(x, Wq, Wk, Wv, Wo):
    x = np.asarray(x, dtype=np.float32)
    Wq = np.asarray(Wq, dtype=np.float16)
    Wk = np.asarray(Wk, dtype=np.float16)
    Wv = np.asarray(Wv, dtype=np.float16)
    Wo = np.asarray(Wo, dtype=np.float16)
    xTs = [np.ascontiguousarray(x[b].T.astype(np.float16)) for b in range(2)]
    iden = np.eye(128, dtype=np.float16)
    ones = np.ones((128, 1), dtype=np.float16)
    maps = []
    for core in range(8):
        b, h = divmod(core, 4)
        maps.append({
            "xT": xTs[b],
            "wqT": np.ascontiguousarray(Wq[DQ * h:DQ * (h + 1), :].T),
            "wkT": np.ascontiguousarray(Wk[HD * h:HD * (h + 1), :].T),
            "wvT": np.ascontiguousarray(Wv[HD * h:HD * (h + 1), :].T),
            "woT": np.ascontiguousarray(Wo[:, DQ * h:DQ * (h + 1)].T),
            "iden": iden,
            "ones": ones,
        })
    return maps


def run(x, Wq, Wk, Wv, Wo, **spmd_kwargs):
    """Build/compile (cached) and run; returns BassKernelResults."""
    global _NC
    if _NC is None:
        _NC = _build()
    from concourse.bass_utils import run_bass_kernel_spmd
    return run_bass_kernel_spmd(_NC, _in_maps(x, Wq, Wk, Wv, Wo),
                                list(range(8)), **spmd_kwargs)


def kernel(x, attn_mask=None, is_causal=None, Wq=None, Wk=None, Wv=None,
           Wo=None, **_ignored):
    res = run(x, Wq, Wk, Wv, Wo)
    y = np.zeros((2, N, EMB), dtype=np.float32)
    for core in range(8):
        y[core // 4] += res.results[core]["y"].astype(np.float32)
    return y


# revision 6
# speedup vs baseline: 1.0895x; 1.0895x over previous
"""GQA attention kernel for 8 trn2 NeuronCores.

Sharding: core = (b, h) with b = core//4 (batch), h = core%4 (kv head).
Each core handles q heads 4h..4h+3 (a contiguous 512-column block of Wq),
its own kv head (128 rows of Wk/Wv), and the matching 512-column slice of
Wo.  Per-core output is a partial y (row-parallel Wo); host sums the 4
fp16 partials per batch in fp32.

All matmuls run in fp16 (full-rate, half the SBUF/DMA traffic of fp32r)
with fp32 PSUM accumulation, moving dim 1024 (2 PSUM banks) to halve the
instruction/LDWEIGHTS count.  Softmax normalization uses a ones-matmul
partition reduce + gpsimd broadcast + DVE approx reciprocal so neither
the PE nor the exp-loaded scalar engine stalls.
"""

import numpy as np

EMB = 2048
N = 2048          # sequence length
HD = 128          # head dim
NHC = 4           # q heads per core
DQ = NHC * HD     # 512: per-core q concat dim
EC = 16           # e chunks of 128
SC = 16           # s chunks of 128
NB = 512          # n block in k/v projection phase
M = 1024          # n block in q/attention phase
SCALE = 1.0 / np.sqrt(HD)

_NC = None


def _build():
    import concourse.bass as bass
    from concourse import bacc
    import concourse.mybir as mybir
    import concourse.tile as tile
    from concourse.bass import ts

    FP32 = mybir.dt.float32
    F16 = mybir.dt.float16
    P = 128

    nc = bacc.Bacc("TRN2", target_bir_lowering=False, debug=False, num_devices=8)
    xT = nc.declare_dram_parameter("xT", [EMB, N], F16, isOutput=False)
    wqT = nc.declare_dram_parameter("wqT", [EMB, DQ], F16, isOutput=False)
    wkT = nc.declare_dram_parameter("wkT", [EMB, HD], F16, isOutput=False)
    wvT = nc.declare_dram_parameter("wvT", [EMB, HD], F16, isOutput=False)
    woT = nc.declare_dram_parameter("woT", [DQ, EMB], F16, isOutput=False)
    iden_d = nc.declare_dram_parameter("iden", [128, 128], F16, isOutput=False)
    ones_d = nc.declare_dram_parameter("ones", [128, 1], F16, isOutput=False)
    y = nc.declare_dram_parameter("y", [N, EMB], F16, isOutput=True)

    xT_r = xT[:].rearrange("(c p) n -> p c n", p=P)      # (128, 16, 2048)
    wqT_r = wqT[:].rearrange("(c p) d -> p c d", p=P)    # (128, 16, 512)
    wkT_r = wkT[:].rearrange("(c p) d -> p c d", p=P)    # (128, 16, 128)
    wvT_r = wvT[:].rearrange("(c p) d -> p c d", p=P)
    woT_r = woT[:].rearrange("(c p) e -> p c e", p=P)    # (128, 4, 2048)

    with tile.TileContext(nc) as tc:
      with tc.tile_pool(name="consts", bufs=1) as consts, \
           tc.tile_pool(name="persist", bufs=1) as persist:
        identity = consts.tile([P, P], F16, tag="identity")
        ones = consts.tile([P, 1], F16, tag="ones")
        # small weights first so the k projection can start right away
        wk = persist.tile([P, EC, HD], F16, tag="wk")
        wv = persist.tile([P, EC, HD], F16, tag="wv")
        nc.sync.dma_start(wk[:], wkT_r)
        nc.sync.dma_start(wv[:], wvT_r)
        nc.sync.dma_start(identity[:], iden_d[:])
        nc.sync.dma_start(ones[:], ones_d[:])

        # full x stays resident in SBUF (64 KiB/partition); DMA'd in
        # n-chunks so the k projection can start after the first chunk
        xt = persist.tile([P, EC, N], F16, tag="xt")
        for nb in range(N // NB):
            nc.sync.dma_start(xt[:, :, ts(nb, NB)], xT_r[:, :, ts(nb, NB)])

        wq = persist.tile([P, EC, DQ], F16, tag="wq")
        nc.sync.dma_start(wq[:, :, 0:256], wqT_r[:, :, 0:256])
        nc.sync.dma_start(wq[:, :, 256:512], wqT_r[:, :, 256:512])
        wo = persist.tile([P, NHC, EMB], F16, tag="wo")
        nc.sync.dma_start(wo[:], woT_r)

        kT = persist.tile([P, N], F16, tag="kT")
        V = persist.tile([P, SC, HD], F16, tag="V")
        qT = [persist.tile([P, N], F16, tag=f"qT{g}", name=f"qT{g}")
              for g in range(NHC)]
        OT = [persist.tile([P, N], F16, tag=f"OT{g}", name=f"OT{g}")
              for g in range(NHC)]

        # ---------------- k/v projections ----------------
        with tc.tile_pool(name="vTp", bufs=1) as vTp, \
             tc.tile_pool(name="psA", bufs=4, space="PSUM") as psA, \
             tc.tile_pool(name="psT", bufs=2, space="PSUM") as psT:
            vT = vTp.tile([P, N], F16, tag="vT")
            for nb in range(N // NB):
                nsl = ts(nb, NB)
                for t in range(2):
                    ps = psA.tile([P, NB], FP32, tag="psA", name=f"psKV_{nb}_{t}")
                    w = wk if t == 0 else wv
                    for e in range(EC):
                        nc.tensor.matmul(
                            ps[:], w[:, e, :], xt[:, e, nsl],
                            start=(e == 0), stop=(e == EC - 1),
                        )
                    if t == 0:
                        nc.scalar.copy(kT[:, nsl], ps[:])
                    else:
                        nc.scalar.copy(vT[:, nsl], ps[:])
                # transpose the 4 freshly-written vT s-chunks into V
                for j in range(nb * 4, nb * 4 + 4):
                    pt = psT.tile([P, P], F16, tag="psT", name=f"psT_{j}")
                    nc.tensor.transpose(pt[:], vT[:, ts(j, P)], identity[:])
                    nc.scalar.copy(V[:, j, :], pt[:])

        # ---------------- per-head: q proj then attention ----------------
        with tc.tile_pool(name="esp", bufs=3) as esp, \
             tc.tile_pool(name="lap", bufs=2) as lap, \
             tc.tile_pool(name="rp", bufs=2) as rp, \
             tc.tile_pool(name="rbp", bufs=2) as rbp, \
             tc.tile_pool(name="psS", bufs=2, space="PSUM") as psS, \
             tc.tile_pool(name="psO", bufs=2, space="PSUM") as psO:
            for g in range(NHC):
                # q projection for head g (2 n-blocks of 1024 x 16 e-chunks);
                # shares the psS buffers (same tag/shape) to fit PSUM
                for nb in range(N // M):
                    ps = psS.tile([P, M], FP32, tag="psS", name=f"psQ_{g}_{nb}")
                    for e in range(EC):
                        for u in range(2):
                            nc.tensor.matmul(
                                ps[:, ts(u, NB)],
                                wq[:, e, ts(g, HD)],
                                xt[:, e, ts(2 * nb + u, NB)],
                                start=(e == 0), stop=(e == EC - 1),
                            )
                    nc.vector.tensor_copy(qT[g][:, ts(nb, M)], ps[:])

                # attention for head g over two n-halves
                for m in range(2):
                    msl = ts(m, M)
                    lacc = lap.tile([P, M], F16, tag="lacc",
                                    name=f"lacc_{g}_{m}")
                    ot_ps = psO.tile([P, M], FP32, tag="psO",
                                     name=f"psO_{g}_{m}")
                    for j in range(SC):
                        s_ps = psS.tile([P, M], FP32, tag="psS",
                                        name=f"psS_{g}_{m}_{j}")
                        for u in range(2):
                            nc.tensor.matmul(
                                s_ps[:, ts(u, NB)],
                                kT[:, ts(j, P)],
                                qT[g][:, ts(2 * m + u, NB)],
                                start=True, stop=True,
                            )
                        es = esp.tile([P, M], F16, tag="es",
                                      name=f"es_{g}_{m}_{j}")
                        nc.scalar.activation(
                            es[:], s_ps[:],
                            mybir.ActivationFunctionType.Exp,
                            scale=float(SCALE),
                        )
                        if j == 0:
                            nc.vector.tensor_copy(lacc[:], es[:])
                        else:
                            nc.vector.tensor_add(lacc[:], lacc[:], es[:])
                        for u in range(2):
                            nc.tensor.matmul(
                                ot_ps[:, ts(u, NB)], V[:, j, :],
                                es[:, ts(u, NB)],
                                start=(j == 0), stop=(j == SC - 1),
                            )
                    # partition-reduce lacc via ones-matmul -> (1, M), then
                    # broadcast and approx-reciprocal at full DVE width
                    psl = psS.tile([1, M], FP32, tag="psS",
                                   name=f"psl_{g}_{m}")
                    for u in range(2):
                        nc.tensor.matmul(psl[:, ts(u, NB)], ones[:, 0:1],
                                         lacc[:, ts(u, NB)],
                                         start=True, stop=True)
                    r_t = rp.tile([1, M], FP32, tag="r", name=f"r_{g}_{m}")
                    nc.scalar.copy(r_t[:], psl[:])
                    rb = rbp.tile([P, M], FP32, tag="rb", name=f"rb_{g}_{m}")
                    nc.gpsimd.partition_broadcast(rb[:], r_t[:])
                    nc.vector.reciprocal_approx_fast(rb[:], rb[:])
                    nc.vector.tensor_mul(OT[g][:, msl], ot_ps[:], rb[:])

        # ---------------- output projection ----------------
        with tc.tile_pool(name="yep", bufs=2) as yep, \
             tc.tile_pool(name="psC", bufs=2, space="PSUM") as psC:
            for nt in range(N // P):
                yp = psC.tile([P, EMB], FP32, tag="psC", name=f"psC_{nt}")
                for g in range(NHC):
                    lhsT = OT[g][:, ts(nt, P)]
                    for ob in range(4):
                        nc.tensor.matmul(
                            yp[:, ts(ob, NB)],
                            lhsT,
                            wo[:, g, ts(ob, NB)],
                            start=(g == 0), stop=(g == NHC - 1),
                        )
                ysb = yep.tile([P, EMB], F16, tag="ysb", name=f"ysb_{nt}")
                nc.scalar.copy(ysb[:], yp[:])
                nc.sync.dma_start(y[ts(nt, P), :], ysb[:])

    nc.compile()
    return nc


def _in_maps(x, Wq, Wk, Wv, Wo):
    x = np.asarray(x, dtype=np.float32)
    Wq = np.asarray(Wq, dtype=np.float16)
    Wk = np.asarray(Wk, dtype=np.float16)
    Wv = np.asarray(Wv, dtype=np.float16)
    Wo = np.asarray(Wo, dtype=np.float16)
    xTs = [np.ascontiguousarray(x[b].T.astype(np.float16)) for b in range(2)]
    iden = np.eye(128, dtype=np.float16)
    ones = np.ones((128, 1), dtype=np.float16)
    maps = []
    for core in range(8):
        b, h = divmod(core, 4)
        maps.append({
            "xT": xTs[b],
            "wqT": np.ascontiguousarray(Wq[DQ * h:DQ * (h + 1), :].T),
            "wkT": np.ascontiguousarray(Wk[HD * h:HD * (h + 1), :].T),
            "wvT": np.ascontiguousarray(Wv[HD * h:HD * (h + 1), :].T),
            "woT": np.ascontiguousarray(Wo[:, DQ * h:DQ * (h + 1)].T),
            "iden": iden,
            "ones": ones,
        })
    return maps


def run(x, Wq, Wk, Wv, Wo, **spmd_kwargs):
    """Build/compile (cached) and run; returns BassKernelResults."""
    global _NC
    if _NC is None:
        _NC = _build()
    from concourse.bass_utils import run_bass_kernel_spmd
    return run_bass_kernel_spmd(_NC, _in_maps(x, Wq, Wk, Wv, Wo),
                                list(range(8)), **spmd_kwargs)


def kernel(x, attn_mask=None, is_causal=None, Wq=None, Wk=None, Wv=None,
           Wo=None, **_ignored):
    res = run(x, Wq, Wk, Wv, Wo)
    y = np.zeros((2, N, EMB), dtype=np.float32)
    for core in range(8):
        y[core // 4] += res.results[core]["y"].astype(np.float32)
    return y


# revision 7
# speedup vs baseline: 1.2189x; 1.1188x over previous
"""GQA attention kernel for 8 trn2 NeuronCores.

Sharding: core = (b, h) with b = core//4 (batch), h = core%4 (kv head).
Each core handles q heads 4h..4h+3 (a contiguous 512-column block of Wq),
its own kv head (128 rows of Wk/Wv), and the matching 512-column slice of
Wo.  Per-core output is a partial y (row-parallel Wo); host sums the 4
fp16 partials per batch in fp32.

All matmuls run in fp16 (full-rate at 2.4 GHz) with fp32 PSUM
accumulation.  The attention j-loop is exp-bound on the scalar engine,
so the q projection of head g+1 is software-pipelined into head g's
attention loop (one projection matmul per j iteration) to keep the PE
busy during exp waits.  Softmax normalization: an all-ones [128,128]
matmul gives the partition-broadcast key sum in one PE op, followed by
a full-width DVE approx reciprocal and multiply.
"""

import numpy as np

EMB = 2048
N = 2048          # sequence length
HD = 128          # head dim
NHC = 4           # q heads per core
DQ = NHC * HD     # 512: per-core q concat dim
EC = 16           # e chunks of 128
SC = 16           # s chunks of 128
NB = 512          # n block size everywhere
NQ = 4            # n quarters in attention phase
SCALE = 1.0 / np.sqrt(HD)

_NC = None


def _build():
    import concourse.bass as bass
    from concourse import bacc
    import concourse.mybir as mybir
    import concourse.tile as tile
    from concourse.bass import ts

    FP32 = mybir.dt.float32
    F16 = mybir.dt.float16
    P = 128

    nc = bacc.Bacc("TRN2", target_bir_lowering=False, debug=False, num_devices=8)
    xT = nc.declare_dram_parameter("xT", [EMB, N], F16, isOutput=False)
    wqT = nc.declare_dram_parameter("wqT", [EMB, DQ], F16, isOutput=False)
    wkT = nc.declare_dram_parameter("wkT", [EMB, HD], F16, isOutput=False)
    wvT = nc.declare_dram_parameter("wvT", [EMB, HD], F16, isOutput=False)
    woT = nc.declare_dram_parameter("woT", [DQ, EMB], F16, isOutput=False)
    iden_d = nc.declare_dram_parameter("iden", [128, 128], F16, isOutput=False)
    ones_d = nc.declare_dram_parameter("ones", [128, 128], F16, isOutput=False)
    y = nc.declare_dram_parameter("y", [N, EMB], F16, isOutput=True)

    xT_r = xT[:].rearrange("(c p) n -> p c n", p=P)      # (128, 16, 2048)
    wqT_r = wqT[:].rearrange("(c p) d -> p c d", p=P)    # (128, 16, 512)
    wkT_r = wkT[:].rearrange("(c p) d -> p c d", p=P)    # (128, 16, 128)
    wvT_r = wvT[:].rearrange("(c p) d -> p c d", p=P)
    woT_r = woT[:].rearrange("(c p) e -> p c e", p=P)    # (128, 4, 2048)

    with tile.TileContext(nc) as tc:
      with tc.tile_pool(name="consts", bufs=1) as consts, \
           tc.tile_pool(name="persist", bufs=1) as persist:
        identity = consts.tile([P, P], F16, tag="identity")
        allones = consts.tile([P, P], F16, tag="allones")
        xt = persist.tile([P, EC, N], F16, tag="xt")
        wk = persist.tile([P, EC, HD], F16, tag="wk")
        wv = persist.tile([P, EC, HD], F16, tag="wv")
        # x chunk 0 + wk first: the k projection starts as soon as both land
        nc.sync.dma_start(xt[:, :, ts(0, NB)], xT_r[:, :, ts(0, NB)])
        nc.sync.dma_start(wk[:], wkT_r)
        nc.sync.dma_start(identity[:], iden_d[:])
        nc.sync.dma_start(allones[:], ones_d[:])
        nc.sync.dma_start(wv[:], wvT_r)
        for nb in range(1, N // NB):
            nc.sync.dma_start(xt[:, :, ts(nb, NB)], xT_r[:, :, ts(nb, NB)])

        wq = persist.tile([P, EC, DQ], F16, tag="wq")
        nc.sync.dma_start(wq[:, :, 0:256], wqT_r[:, :, 0:256])
        nc.sync.dma_start(wq[:, :, 256:512], wqT_r[:, :, 256:512])
        wo = persist.tile([P, NHC, EMB], F16, tag="wo")
        nc.sync.dma_start(wo[:], woT_r)

        kT = persist.tile([P, N], F16, tag="kT")
        V = persist.tile([P, SC, HD], F16, tag="V")
        qT = [persist.tile([P, N], F16, tag=f"qT{g}", name=f"qT{g}")
              for g in range(NHC)]
        OT = [persist.tile([P, N], F16, tag=f"OT{g}", name=f"OT{g}")
              for g in range(NHC)]

        # ---------------- k/v projections ----------------
        with tc.tile_pool(name="vTp", bufs=1) as vTp, \
             tc.tile_pool(name="psA", bufs=4, space="PSUM") as psA, \
             tc.tile_pool(name="psT", bufs=2, space="PSUM") as psT:
            vT = vTp.tile([P, N], F16, tag="vT")
            for nb in range(N // NB):
                nsl = ts(nb, NB)
                for t in range(2):
                    ps = psA.tile([P, NB], FP32, tag="psA", name=f"psKV_{nb}_{t}")
                    w = wk if t == 0 else wv
                    for e in range(EC):
                        nc.tensor.matmul(
                            ps[:], w[:, e, :], xt[:, e, nsl],
                            start=(e == 0), stop=(e == EC - 1),
                        )
                    if t == 0:
                        nc.scalar.copy(kT[:, nsl], ps[:])
                    else:
                        nc.scalar.copy(vT[:, nsl], ps[:])
                # transpose the 4 freshly-written vT s-chunks into V
                for j in range(nb * 4, nb * 4 + 4):
                    pt = psT.tile([P, P], F16, tag="psT", name=f"psT_{j}")
                    nc.tensor.transpose(pt[:], vT[:, ts(j, P)], identity[:])
                    nc.scalar.copy(V[:, j, :], pt[:])

        # ------------- attention with pipelined q projection -------------
        with tc.tile_pool(name="esp", bufs=3) as esp, \
             tc.tile_pool(name="lap", bufs=2) as lap, \
             tc.tile_pool(name="rbp", bufs=2) as rbp, \
             tc.tile_pool(name="psS", bufs=3, space="PSUM") as psS, \
             tc.tile_pool(name="psO", bufs=2, space="PSUM") as psO, \
             tc.tile_pool(name="psQ", bufs=2, space="PSUM") as psQ:

            def qproj_step(g, jj, ps_box):
                """One matmul of head g's q projection (jj in 0..63)."""
                nb, e = divmod(jj, EC)
                if e == 0:
                    ps_box[0] = psQ.tile([P, NB], FP32, tag="psQ",
                                         name=f"psQ_{g}_{nb}")
                nc.tensor.matmul(
                    ps_box[0][:], wq[:, e, ts(g, HD)], xt[:, e, ts(nb, NB)],
                    start=(e == 0), stop=(e == EC - 1),
                )
                if e == EC - 1:
                    nc.vector.tensor_copy(qT[g][:, ts(nb, NB)], ps_box[0][:])

            qbox = [None]
            for jj in range(NQ * SC):      # head 0 q projection, unpipelined
                qproj_step(0, jj, qbox)

            for g in range(NHC):
                for m in range(NQ):
                    msl = ts(m, NB)
                    lacc = lap.tile([P, NB], F16, tag="lacc",
                                    name=f"lacc_{g}_{m}")
                    ot_ps = psO.tile([P, NB], FP32, tag="psO",
                                     name=f"psO_{g}_{m}")
                    for j in range(SC):
                        s_ps = psS.tile([P, NB], FP32, tag="psS",
                                        name=f"psS_{g}_{m}_{j}")
                        nc.tensor.matmul(
                            s_ps[:], kT[:, ts(j, P)], qT[g][:, msl],
                            start=True, stop=True,
                        )
                        if g < NHC - 1:
                            qproj_step(g + 1, m * SC + j, qbox)
                        es = esp.tile([P, NB], F16, tag="es",
                                      name=f"es_{g}_{m}_{j}")
                        nc.scalar.activation(
                            es[:], s_ps[:],
                            mybir.ActivationFunctionType.Exp,
                            scale=float(SCALE),
                        )
                        if j == 0:
                            nc.vector.tensor_copy(lacc[:], es[:])
                        else:
                            nc.vector.tensor_add(lacc[:], lacc[:], es[:])
                        nc.tensor.matmul(
                            ot_ps[:], V[:, j, :], es[:],
                            start=(j == 0), stop=(j == SC - 1),
                        )
                    # all-ones matmul: every partition gets the key-sum of
                    # lacc -> reciprocal + normalize at full DVE width
                    psl = psS.tile([P, NB], FP32, tag="psS",
                                   name=f"psl_{g}_{m}")
                    nc.tensor.matmul(psl[:], allones[:], lacc[:],
                                     start=True, stop=True)
                    rb = rbp.tile([P, NB], FP32, tag="rb", name=f"rb_{g}_{m}")
                    nc.vector.reciprocal_approx_fast(rb[:], psl[:])
                    nc.vector.tensor_mul(OT[g][:, msl], ot_ps[:], rb[:])

        # ---------------- output projection ----------------
        with tc.tile_pool(name="yep", bufs=2) as yep, \
             tc.tile_pool(name="psC", bufs=2, space="PSUM") as psC:
            for nt in range(N // P):
                yp = psC.tile([P, EMB], FP32, tag="psC", name=f"psC_{nt}")
                for g in range(NHC):
                    lhsT = OT[g][:, ts(nt, P)]
                    for ob in range(4):
                        nc.tensor.matmul(
                            yp[:, ts(ob, NB)],
                            lhsT,
                            wo[:, g, ts(ob, NB)],
                            start=(g == 0), stop=(g == NHC - 1),
                        )
                ysb = yep.tile([P, EMB], F16, tag="ysb", name=f"ysb_{nt}")
                nc.scalar.copy(ysb[:], yp[:])
                nc.sync.dma_start(y[ts(nt, P), :], ysb[:])

    nc.compile()
    return nc


def _in_maps(x, Wq, Wk, Wv, Wo):
    x = np.asarray(x, dtype=np.float32)
    Wq = np.asarray(Wq, dtype=np.float16)
    Wk = np.asarray(Wk, dtype=np.float16)
    Wv = np.asarray(Wv, dtype=np.float16)
    Wo = np.asarray(Wo, dtype=np.float16)
    xTs = [np.ascontiguousarray(x[b].T.astype(np.float16)) for b in range(2)]
    iden = np.eye(128, dtype=np.float16)
    ones = np.ones((128, 128), dtype=np.float16)
    maps = []
    for core in range(8):
        b, h = divmod(core, 4)
        maps.append({
            "xT": xTs[b],
            "wqT": np.ascontiguousarray(Wq[DQ * h:DQ * (h + 1), :].T),
            "wkT": np.ascontiguousarray(Wk[HD * h:HD * (h + 1), :].T),
            "wvT": np.ascontiguousarray(Wv[HD * h:HD * (h + 1), :].T),
            "woT": np.ascontiguousarray(Wo[:, DQ * h:DQ * (h + 1)].T),
            "iden": iden,
            "ones": ones,
        })
    return maps


def run(x, Wq, Wk, Wv, Wo, **spmd_kwargs):
    """Build/compile (cached) and run; returns BassKernelResults."""
    global _NC
    if _NC is None:
        _NC = _build()
    from concourse.bass_utils import run_bass_kernel_spmd
    return run_bass_kernel_spmd(_NC, _in_maps(x, Wq, Wk, Wv, Wo),
                                list(range(8)), **spmd_kwargs)


def kernel(x, attn_mask=None, is_causal=None, Wq=None, Wk=None, Wv=None,
           Wo=None, **_ignored):
    res = run(x, Wq, Wk, Wv, Wo)
    y = np.zeros((2, N, EMB), dtype=np.float32)
    for core in range(8):
        y[core // 4] += res.results[core]["y"].astype(np.float32)
    return y
